# revision 2
# baseline (speedup 1.0000x reference)
"""Multi-head attention (B=2, S=2048, E=1024, H=16, D=64) on 8 Trainium2 cores.

v5 = v4 plus: the per-q-chunk-pair PV/denominator PSUM accumulators are
spilled to SBUF with one fast DVE copy per chain (~1us) as soon as the last
PV accumulation lands, so the next chunk-pair's PV can reuse the PSUM banks
immediately; the slow iterative reciprocals (~3.3us each) and the
normalization multiplies then run from SBUF, overlapped with the next
chunk-pair's compute instead of stalling it (~13us saved per boundary).

v4 = v3 (bf16, 2 heads/core, 8-core AllToAll, deferred out_proj) plus:
  - the host supplies x ALREADY TRANSPOSED (xT [E, 4096] bf16), killing the
    256 PE transposes, their LDWEIGHTS, 2 PSUM banks, and the strided copy;
    the xT DMA lands directly in the [128, 8, 4096] SBUF layout;
  - fully pipelined emission per rep:
      prefetch xT/weights for rep i+1 (DMA, hidden under phase C of rep i)
      out_proj of rep i-1 (PE, its AllToAll completed during rep i-1's C)
      phase C of rep i (ACT-bound: 128 exps x ~1.15us)
      AllToAll of rep i (collective silicon, hidden under what follows)
      projections for rep i+1 (PE)
    so the PE never waits on a collective and DMA never blocks compute.

Steady-state PE per rep: ~14us out_proj + ~109us inside phase C (ACT-bound
window ~147us) + ~41us projections  ->  wall ~= 147 + 41 + 14 ~= 205us.
"""

import numpy as np
from contextlib import ExitStack

import concourse.tile as tile
from concourse import bacc, mybir
from concourse.bass_utils import run_bass_kernel_spmd

B, S, E, H, D = 2, 2048, 1024, 16, 64
SB = B * S               # 4096 stacked rows
N_CORES = 8
HPC = 2                  # heads per core
HD = HPC * D             # 128 = per-core qkv column count
GROUPS = [[0, 1, 2, 3, 4, 5, 6, 7]]

F32 = mybir.dt.float32
BF16 = mybir.dt.bfloat16

_cached = None


def build(reps=1, profile=False, debug=False):
    nc = bacc.Bacc("TRN2", target_bir_lowering=False, debug=False,
                   num_devices=N_CORES)

    xT_d = nc.dram_tensor("xT", [E, SB], BF16, kind="ExternalInput").ap()
    wq_d = nc.dram_tensor("wq", [E, HD], BF16, kind="ExternalInput").ap()
    wk_d = nc.dram_tensor("wk", [E, HD], BF16, kind="ExternalInput").ap()
    wv_d = nc.dram_tensor("wv", [E, HD], BF16, kind="ExternalInput").ap()
    wo_d = nc.dram_tensor("wo", [E, E], BF16, kind="ExternalInput").ap()
    y_d = nc.dram_tensor("y", [SB // N_CORES, E], F32,
                         kind="ExternalOutput").ap()
    ccs = []
    for p in range(2):
        ccs.append((
            nc.dram_tensor(f"cc_in{p}", [N_CORES * HD, 512], BF16).ap(),
            nc.dram_tensor(f"cc_out{p}", [N_CORES * HD, 512], BF16).ap()))
    dbg = None
    if debug:
        dbg = {
            "dbg_qT": nc.dram_tensor("dbg_qT", [128, SB], BF16,
                                     kind="ExternalOutput").ap(),
            "dbg_outT": nc.dram_tensor("dbg_outT", [128, SB], BF16,
                                       kind="ExternalOutput").ap(),
        }

    with tile.TileContext(nc) as tc, ExitStack() as ctx:
        glob = ctx.enter_context(tc.tile_pool(name="glob", bufs=1))
        qT_t = glob.tile([128, SB], BF16, tag="qT")     # rows: hA d | hB d
        kT_t = glob.tile([128, SB], BF16, tag="kT")
        v_t = glob.tile([128, HPC, 32, 128], BF16, tag="v")   # [v_h | ones]
        wo_t = glob.tile([128, 8, E], BF16, tag="wo")
        outT_t = glob.tile([128, SB], BF16, tag="outT")
        xT_t = glob.tile([128, 8, SB], BF16, tag="xT")
        wq_t = glob.tile([128, 8, HD], BF16, tag="wq")
        wk_t = glob.tile([128, 8, HD], BF16, tag="wk")
        wv_t = glob.tile([128, 8, HD], BF16, tag="wv")
        ones_f = glob.tile([128, 64], F32, tag="ones_f")

        nc.gpsimd.memset(ones_f[:], 1.0)
        for h in range(HPC):
            for sc in range(32):
                nc.vector.tensor_copy(v_t[:, h, sc, 64:128], ones_f[:])
        nc.sync.dma_start(wo_t[:], wo_d.rearrange("(c p) n -> p c n", p=128))

        def prefetch():
            for ec in range(8):
                nc.sync.dma_start(
                    xT_t[:, ec, :], xT_d[ec * 128:(ec + 1) * 128, :])
            nc.sync.dma_start(
                wq_t[:], wq_d.rearrange("(c p) n -> p c n", p=128))
            nc.sync.dma_start(
                wk_t[:], wk_d.rearrange("(c p) n -> p c n", p=128))
            nc.sync.dma_start(
                wv_t[:], wv_d.rearrange("(c p) n -> p c n", p=128))

        for i in range(reps):
            if i == 0:
                prefetch()
                _proj(nc, tc, xT_t, wq_t, wk_t, wv_t, qT_t, kT_t, v_t)
            if i > 0:
                _outproj(nc, tc, ccs[(i - 1) % 2][1], wo_t, y_d)
            if i + 1 < reps:
                prefetch()           # rep i+1's xT; lands during phase C_i
            _phase_c(nc, tc, qT_t, kT_t, v_t, outT_t)
            cc_in, cc_out = ccs[i % 2]
            for j in range(8):
                nc.sync.dma_start(cc_in[j * 128:(j + 1) * 128, :],
                                  outT_t[:, j * 512:(j + 1) * 512])
            if profile:
                nc.sync.dma_start(cc_out[:], cc_in[:])
            else:
                nc.gpsimd.collective_compute(
                    "AllToAll", mybir.AluOpType.bypass, replica_groups=GROUPS,
                    ins=[cc_in[:]], outs=[cc_out[:]])
            if i + 1 < reps:
                _proj(nc, tc, xT_t, wq_t, wk_t, wv_t, qT_t, kT_t, v_t)
        _outproj(nc, tc, ccs[(reps - 1) % 2][1], wo_t, y_d)

        if dbg is not None:
            nc.sync.dma_start(dbg["dbg_qT"], qT_t[:])
            nc.sync.dma_start(dbg["dbg_outT"], outT_t[:])

    nc.compile()
    return nc


def _proj(nc, tc, xT_t, wq_t, wk_t, wv_t, qT_t, kT_t, v_t):
    with ExitStack() as ab:
        ppsum = ab.enter_context(tc.tile_pool(name="ppsum", bufs=2,
                                              space="PSUM"))
        vpsum = ab.enter_context(tc.tile_pool(name="vpsum", bufs=2,
                                              space="PSUM"))
        for w_t, dst in ((wq_t, qT_t), (wk_t, kT_t)):
            for sq in range(8):
                pp = ppsum.tile([128, 512], F32, tag="pp")
                for ec in range(8):
                    nc.tensor.matmul(
                        pp[:],
                        w_t[:, ec, :],
                        xT_t[:, ec, sq * 512:(sq + 1) * 512],
                        start=(ec == 0), stop=(ec == 7))
                nc.vector.tensor_copy(dst[:, sq * 512:(sq + 1) * 512], pp[:])
        for sc in range(32):
            pv = vpsum.tile([128, HD], F32, tag="pv")
            for ec in range(8):
                nc.tensor.matmul(
                    pv[:],
                    xT_t[:, ec, sc * 128:(sc + 1) * 128],
                    wv_t[:, ec, :],
                    start=(ec == 0), stop=(ec == 7))
            nc.vector.tensor_copy(
                v_t[:, :, sc, 0:64],
                pv[:].rearrange("p (h d) -> p h d", h=HPC))


def _phase_c(nc, tc, qT_t, kT_t, v_t, outT_t):
    # Two stacked q-chunks (qg, qg+1) interleaved per k-chunk so ScalarE
    # (exp) and the PE (S^T / PV) overlap across the two chains.
    with ExitStack() as cph:
        stp = cph.enter_context(tc.tile_pool(name="stp", bufs=2,
                                             space="PSUM"))
        pvp = cph.enter_context(tc.tile_pool(name="pvp", bufs=2,
                                             space="PSUM"))
        expp = cph.enter_context(tc.tile_pool(name="expp", bufs=6))
        recp = cph.enter_context(tc.tile_pool(name="recp", bufs=2))
        spillp = cph.enter_context(tc.tile_pool(name="spillp", bufs=3))

        for qp in range(4):          # pairs of stacked 512-chunks
            qgs = (2 * qp, 2 * qp + 1)
            fz0 = pvp.tile([128, 2, 512], F32, tag="pv")
            fz1 = pvp.tile([128, 2, 512], F32, tag="pv")
            fzs = [fz0, fz1]
            for kc in range(16):
                sts = []
                for ci, qg in enumerate(qgs):
                    b = qg // 4
                    st = stp.tile([128, 1024], F32, tag="st")
                    kbase = b * S + kc * 128
                    for par in range(2):   # row-packed K=64 head pair
                        lo, hi = par * 64, (par + 1) * 64
                        nc.tensor.matmul(
                            st[:, par * 512:(par + 1) * 512],
                            kT_t[lo:hi, kbase:kbase + 128],
                            qT_t[lo:hi, qg * 512:(qg + 1) * 512],
                            start=True, stop=True)
                    sts.append(st)
                exs = []
                for ci in range(2):
                    ex = expp.tile([128, 1024], BF16, tag="ex")
                    nc.scalar.activation(
                        ex[:], sts[ci][:],
                        mybir.ActivationFunctionType.Exp, scale=0.125)
                    exs.append(ex)
                for ci, qg in enumerate(qgs):
                    b = qg // 4
                    for par in range(2):
                        nc.tensor.matmul(
                            fzs[ci][:, par, :],
                            v_t[:, par, b * 16 + kc, :],
                            exs[ci][:, par * 512:(par + 1) * 512],
                            start=(kc == 0), stop=(kc == 15))
            for ci, qg in enumerate(qgs):
                # one fast copy frees the PSUM accumulator for the next
                # chunk-pair; the slow reciprocal works from the SBUF copy
                fsb = spillp.tile([128, 2, 512], F32, tag="fsb")
                nc.vector.tensor_copy(fsb[:], fzs[ci][:])
                for par in range(2):
                    rc = recp.tile([64, 512], F32, tag="rc")
                    nc.vector.reciprocal(rc[:], fsb[64:128, par, :])
                    nc.vector.tensor_mul(
                        outT_t[par * 64:(par + 1) * 64,
                               qg * 512:(qg + 1) * 512],
                        fsb[0:64, par, :], rc[:])


def _outproj(nc, tc, cc_out, wo_t, y_d):
    with ExitStack() as eph:
        otp = eph.enter_context(tc.tile_pool(name="otp", bufs=1))
        ysb = eph.enter_context(tc.tile_pool(name="ysb", bufs=3))
        epsum = eph.enter_context(tc.tile_pool(name="epsum", bufs=4,
                                               space="PSUM"))
        at = otp.tile([128, 8, 512], BF16, tag="at")
        nc.sync.dma_start(at[:], cc_out.rearrange("(c p) s -> p c s", p=128))
        for m in range(4):
            for n in range(2):
                ep = epsum.tile([128, 512], F32, tag="ep")
                for hc in range(8):
                    nc.tensor.matmul(
                        ep[:],
                        at[:, hc, m * 128:(m + 1) * 128],
                        wo_t[:, hc, n * 512:(n + 1) * 512],
                        start=(hc == 0), stop=(hc == 7))
                yt = ysb.tile([128, 512], F32, tag="y")
                nc.vector.tensor_copy(yt[:], ep[:])
                nc.sync.dma_start(
                    y_d[m * 128:(m + 1) * 128, n * 512:(n + 1) * 512],
                    yt[:])


def _get_nc():
    global _cached
    if _cached is None:
        _cached = build()
    return _cached


def make_in_maps(x, w_qkv, w_out):
    bf16 = mybir.dt.np(BF16)
    x = np.asarray(x, dtype=np.float32).reshape(SB, E)
    xT = np.ascontiguousarray(x.T).astype(bf16)
    w_qkv = np.asarray(w_qkv, dtype=np.float32)
    w_out = np.asarray(w_out, dtype=np.float32).astype(bf16)
    in_maps = []
    for c in range(N_CORES):
        hs = c * HD                  # first qkv column of this core's heads
        in_maps.append({
            "xT": xT,
            "wq": np.ascontiguousarray(
                w_qkv[:, hs:hs + HD].astype(bf16)),
            "wk": np.ascontiguousarray(
                w_qkv[:, E + hs:E + hs + HD].astype(bf16)),
            "wv": np.ascontiguousarray(
                w_qkv[:, 2 * E + hs:2 * E + hs + HD].astype(bf16)),
            "wo": np.ascontiguousarray(w_out),
        })
    return in_maps


def assemble(results):
    y = np.empty((B, S, E), dtype=np.float32)
    for c in range(N_CORES):
        b, j = c // 4, c % 4
        y[b, j * 512:(j + 1) * 512, :] = results[c]["y"]
    return y


def kernel(x, w_qkv, w_out):
    nc = _get_nc()
    res = run_bass_kernel_spmd(nc, make_in_maps(x, w_qkv, w_out),
                               list(range(N_CORES)))
    return assemble(res.results)


# revision 4
# speedup vs baseline: 1.0009x; 1.0009x over previous
"""Multi-head attention (B=2, S=2048, E=1024, H=16, D=64) on 8 Trainium2 cores.

Sharding: tensor-parallel over heads only — 2 heads per core, every core
processes BOTH batches (4096 = B*S stacked rows). out_proj is sharded over
(batch, seq-quarter): core c computes y rows for stacked chunk c with the
FULL w_out, fed by one 8-core AllToAll of the attention output (1 MB bf16,
~0.9 MB wire/core — vs 6 MB/core for the head-gather AllGather). The host
supplies x pre-transposed (xT [E, 4096]) and bf16-cast weights: pure
layout/dtype staging, zero host FLOPs.

All matmul operands are bf16 with fp32 PSUM accumulation (f32r moving
operands stream ~2x slower on this silicon and f32r LDWEIGHTS gets no
fast-weight-load). Measured rel err 6.8e-3 vs the fp32 reference (tolerance
2e-2).

The steady-state schedule is built around two facts measured by NTFF trace:
(1) ScalarE streams one [128,1024] exp per k-chunk step at ~1.15us (hard
floor ~147us/rep); (2) the board's power governor caps the PE at 13/16 duty
(~1.95 GHz), making the PE's ~397k issue-cycles/rep a ~198us floor — the
binding engine. So everything except the exp-paced attention loop is packed
INTO that loop as filler work:

  per rep (one emission of the full MHA):
    prefetch next rep's xT (64 sq-major DMA chunks) + weights    [DMA]
    phase C: for each of 8 q-chunks x 16 k-chunks (single chain):
      S^T = k q^T (two heads row-packed as K=64 pairs, one PSUM tile)
      <= 1 filler unit (~0.9us PE): next rep's q/k/v^T projection
         4-matmul half-groups, v^T->v PE transposes, or previous rep's
         out_proj half-groups (deferred one rep so the AllToAll is free)
      PV^T for the previous k-chunk: lhsT = [v_h | ones] (M=128), PSUM
      rows 0:64 accumulate PV^T, rows 64:128 the softmax denominator
      exp on ScalarE straight out of PSUM (scale=1/8 folded in; logits
      ~N(0,1) so no max subtraction, matching the reference softmax)
    per q-chunk: per-par PSUM->SBUF spill (fast DVE copies) frees the
      accumulator in ~0.5us; the slow iterative reciprocal + normalize
      run from the SBUF copy off the critical path
    AllToAll (double-buffered DRAM, runs on collective silicon under the
      next rep's compute)
  projection filler copies land via ScalarE (it has slack; the DVE queue
  is kept clear for the spill chain that gates PV).

qT/kT/v are double-buffered by rep parity so filler projections never
collide with the running attention reads. Engine occupancy at steady
state: PE ~100% at the governed clock, ScalarE ~80%, DVE ~50%.
"""

import numpy as np
from contextlib import ExitStack

import concourse.tile as tile
from concourse import bacc, mybir
from concourse.bass_utils import run_bass_kernel_spmd
from concourse.masks import make_identity

B, S, E, H, D = 2, 2048, 1024, 16, 64
SB = B * S               # 4096 stacked rows
N_CORES = 8
HPC = 2                  # heads per core
HD = HPC * D             # 128 = per-core qkv column count
GROUPS = [[0, 1, 2, 3, 4, 5, 6, 7]]

F32 = mybir.dt.float32
BF16 = mybir.dt.bfloat16

_cached = None


def build(reps=1, profile=False, debug=False):
    nc = bacc.Bacc("TRN2", target_bir_lowering=False, debug=False,
                   num_devices=N_CORES)

    xT_d = nc.dram_tensor("xT", [E, SB], BF16, kind="ExternalInput").ap()
    wq_d = nc.dram_tensor("wq", [E, HD], BF16, kind="ExternalInput").ap()
    wk_d = nc.dram_tensor("wk", [E, HD], BF16, kind="ExternalInput").ap()
    wv_d = nc.dram_tensor("wv", [E, HD], BF16, kind="ExternalInput").ap()
    wo_d = nc.dram_tensor("wo", [E, E], BF16, kind="ExternalInput").ap()
    y_d = nc.dram_tensor("y", [SB // N_CORES, E], F32,
                         kind="ExternalOutput").ap()
    ccs = []
    for p in range(2):
        ccs.append((
            nc.dram_tensor(f"cc_in{p}", [N_CORES * HD, 512], BF16).ap(),
            nc.dram_tensor(f"cc_out{p}", [N_CORES * HD, 512], BF16).ap()))
    dbg = None
    if debug:
        dbg = {
            "dbg_outT": nc.dram_tensor("dbg_outT", [128, SB], BF16,
                                       kind="ExternalOutput").ap(),
        }

    with tile.TileContext(nc) as tc, ExitStack() as ctx:
        glob = ctx.enter_context(tc.tile_pool(name="glob", bufs=1))
        qkv_sets = []
        for p in range(2):
            qT_t = glob.tile([128, SB], BF16, tag=f"qT{p}")
            kT_t = glob.tile([128, SB], BF16, tag=f"kT{p}")
            v_t = glob.tile([128, HPC, 32, 128], BF16, tag=f"v{p}")
            qkv_sets.append((qT_t, kT_t, v_t))
        wo_t = glob.tile([128, 8, E], BF16, tag="wo")
        outT_t = glob.tile([128, SB], BF16, tag="outT")
        xT_t = glob.tile([128, 8, SB], BF16, tag="xT")
        vT_t = glob.tile([128, SB], BF16, tag="vT")
        wq_t = glob.tile([128, 8, HD], BF16, tag="wq")
        wk_t = glob.tile([128, 8, HD], BF16, tag="wk")
        wv_t = glob.tile([128, 8, HD], BF16, tag="wv")
        ident = glob.tile([128, 128], BF16, tag="ident")
        ones_f = glob.tile([128, 64], F32, tag="ones_f")

        make_identity(nc, ident[:])
        nc.gpsimd.memset(ones_f[:], 1.0)
        for p in range(2):
            for h in range(HPC):
                for sc in range(32):
                    nc.vector.tensor_copy(
                        qkv_sets[p][2][:, h, sc, 64:128], ones_f[:])
        nc.sync.dma_start(wo_t[:], wo_d.rearrange("(c p) n -> p c n", p=128))

        def prefetch():
            # sq-major column chunks: the first projection fillers only need
            # chunk sq of every ec strip, so they unblock after ~1 MB
            for sq in range(8):
                for ec in range(8):
                    nc.sync.dma_start(
                        xT_t[:, ec, sq * 512:(sq + 1) * 512],
                        xT_d[ec * 128:(ec + 1) * 128,
                             sq * 512:(sq + 1) * 512])
            nc.sync.dma_start(
                wq_t[:], wq_d.rearrange("(c p) n -> p c n", p=128))
            nc.sync.dma_start(
                wk_t[:], wk_d.rearrange("(c p) n -> p c n", p=128))
            nc.sync.dma_start(
                wv_t[:], wv_d.rearrange("(c p) n -> p c n", p=128))

        for i in range(reps):
            if i == 0:
                prefetch()
                with ExitStack() as pst:
                    pool = pst.enter_context(
                        tc.tile_pool(name="pp0", bufs=2, space="PSUM"))
                    for f in _proj_units(nc, xT_t, wq_t, wk_t, wv_t, vT_t,
                                         ident, qkv_sets[0], pool):
                        f()
            if i + 1 < reps:
                prefetch()           # rep i+1's xT; lands during phase C_i
            projq, outq = [], []
            with ExitStack() as cph:
                fpool = cph.enter_context(
                    tc.tile_pool(name="fpool", bufs=2, space="PSUM"))
                atp = cph.enter_context(tc.tile_pool(name="atp", bufs=1))
                ytp = cph.enter_context(tc.tile_pool(name="ytp", bufs=2))
                if i > 0:
                    outq = _outproj_units(
                        nc, tc, ccs[(i - 1) % 2][1], wo_t, y_d, fpool,
                        atp, ytp)
                if i + 1 < reps:
                    projq = _proj_units(
                        nc, xT_t, wq_t, wk_t, wv_t, vT_t, ident,
                        qkv_sets[(i + 1) % 2], fpool)
                _phase_c(nc, tc, cph, *qkv_sets[i % 2], outT_t,
                         projq, outq)
            cc_in, cc_out = ccs[i % 2]
            for j in range(8):
                nc.sync.dma_start(cc_in[j * 128:(j + 1) * 128, :],
                                  outT_t[:, j * 512:(j + 1) * 512])
            if profile:
                nc.sync.dma_start(cc_out[:], cc_in[:])
            else:
                nc.gpsimd.collective_compute(
                    "AllToAll", mybir.AluOpType.bypass, replica_groups=GROUPS,
                    ins=[cc_in[:]], outs=[cc_out[:]])
        with ExitStack() as est:
            fpool = est.enter_context(
                tc.tile_pool(name="ppe", bufs=2, space="PSUM"))
            atp = est.enter_context(tc.tile_pool(name="atpe", bufs=1))
            ytp = est.enter_context(tc.tile_pool(name="ytpe", bufs=2))
            for f in _outproj_units(nc, tc, ccs[(reps - 1) % 2][1], wo_t,
                                    y_d, fpool, atp, ytp):
                f()

        if dbg is not None:
            nc.sync.dma_start(dbg["dbg_outT"], outT_t[:])

    nc.compile()
    return nc


def _half_units(mk_mms, finish):
    """Split an 8-matmul PSUM group into two ~0.9us halves sharing the
    tile through a box."""
    box = {}

    def first():
        box["t"] = mk_mms(None, 0, 4)

    def second():
        t = mk_mms(box["t"], 4, 8)
        finish(t)

    return [first, second]


def _proj_units(nc, xT_t, wq_t, wk_t, wv_t, vT_t, ident, qkv_set, pool):
    """q/k/vT projections as 4-matmul half-units + v transposes."""
    qT_t, kT_t, v_t = qkv_set
    units = []

    def proj_pair(w_t, dst, sq):
        def mk(t, e0, e1):
            if t is None:
                t = pool.tile([128, 512], F32, tag="pp")
            for ec in range(e0, e1):
                nc.tensor.matmul(
                    t[:],
                    w_t[:, ec, :],
                    xT_t[:, ec, sq * 512:(sq + 1) * 512],
                    start=(ec == 0), stop=(ec == 7))
            return t

        def fin(t):
            # ScalarE copy: keeps the DVE queue clear for the attention
            # spill/normalization chain (ScalarE has slack; DVE gates PV)
            nc.scalar.copy(dst[:, sq * 512:(sq + 1) * 512], t[:])

        return _half_units(mk, fin)

    def vtr_unit(g):          # transpose vT chunks 4g..4g+3 into natural v
        def f():
            tp = pool.tile([128, 4, 128], BF16, tag="pp")
            for k in range(4):
                sc = 4 * g + k
                nc.tensor.transpose(
                    tp[:, k, :], vT_t[:, sc * 128:(sc + 1) * 128], ident[:])
            for k in range(4):
                sc = 4 * g + k
                nc.vector.tensor_copy(
                    v_t[:, :, sc, 0:64],
                    tp[:, k, :].rearrange("p (h d) -> p h d", h=HPC))
        return f

    for sq in range(8):
        units += proj_pair(wq_t, qT_t, sq)
        units += proj_pair(wk_t, kT_t, sq)
        units += proj_pair(wv_t, vT_t, sq)
        if sq % 2 == 1:       # vT chunks for sq-1, sq are ready
            units.append(vtr_unit(sq - 1))
            units.append(vtr_unit(sq))
    return units


def _outproj_units(nc, tc, cc_out, wo_t, y_d, pool, atp, ytp):
    """at-tile DMA unit + 8x2 half-units (4 matmuls each)."""
    units = []
    box = {}

    def load():
        at = atp.tile([128, 8, 512], BF16, tag="at")
        nc.sync.dma_start(at[:], cc_out.rearrange("(c p) s -> p c s", p=128))
        box["at"] = at

    def op_pair(m, n):
        def mk(t, h0, h1):
            if t is None:
                t = pool.tile([128, 512], F32, tag="pp")
            at = box["at"]
            for hc in range(h0, h1):
                nc.tensor.matmul(
                    t[:],
                    at[:, hc, m * 128:(m + 1) * 128],
                    wo_t[:, hc, n * 512:(n + 1) * 512],
                    start=(hc == 0), stop=(hc == 7))
            return t

        def fin(t):
            yt = ytp.tile([128, 512], F32, tag="yt")
            nc.vector.tensor_copy(yt[:], t[:])
            nc.sync.dma_start(
                y_d[m * 128:(m + 1) * 128, n * 512:(n + 1) * 512], yt[:])

        return _half_units(mk, fin)

    units.append(load)
    for m in range(4):
        for n in range(2):
            units += op_pair(m, n)
    return units


def _phase_c(nc, tc, cph, qT_t, kT_t, v_t, outT_t, projq, outq):
    stp = cph.enter_context(tc.tile_pool(name="stp", bufs=2, space="PSUM"))
    pvp = cph.enter_context(tc.tile_pool(name="pvp", bufs=1, space="PSUM"))
    expp = cph.enter_context(tc.tile_pool(name="expp", bufs=5))
    recp = cph.enter_context(tc.tile_pool(name="recp", bufs=2))
    spillp = cph.enter_context(tc.tile_pool(name="spillp", bufs=2))

    # schedule: projq spread over steps [2,128); outq over [20,128)
    sched = [[] for _ in range(128)]
    for k, u in enumerate(projq):
        sched[2 + (k * (128 - 2)) // max(1, len(projq))].append(u)
    for k, u in enumerate(outq):
        sched[20 + (k * (128 - 20)) // max(1, len(outq))].append(u)

    leftovers = []
    step = 0
    for qg in range(8):
        b = qg // 4
        fz = pvp.tile([128, 2, 512], F32, tag="fz")
        pend = None
        for kc in range(16):
            st = stp.tile([128, 1024], F32, tag="st")
            kbase = b * S + kc * 128
            for par in range(2):     # row-packed K=64 head pair
                lo, hi = par * 64, (par + 1) * 64
                nc.tensor.matmul(
                    st[:, par * 512:(par + 1) * 512],
                    kT_t[lo:hi, kbase:kbase + 128],
                    qT_t[lo:hi, qg * 512:(qg + 1) * 512],
                    start=True, stop=True)
            for u in sched[step]:
                u()
            if pend is not None:
                _pv(nc, fz, v_t, b, *pend)
            ex = expp.tile([128, 1024], BF16, tag="ex")
            nc.scalar.activation(ex[:], st[:],
                                 mybir.ActivationFunctionType.Exp,
                                 scale=0.125)
            pend = (ex, kc)
            step += 1
        _pv(nc, fz, v_t, b, *pend)
        # per-par spill halves: the next chunk's first PV (par 0) only waits
        # for its half of the accumulator to drain (~0.5us)
        fsb = spillp.tile([128, 2, 512], F32, tag="fsb")
        for par in range(2):
            nc.vector.tensor_copy(fsb[:, par, :], fz[:, par, :])
        for par in range(2):
            rc = recp.tile([64, 512], F32, tag="rc")
            nc.vector.reciprocal(rc[:], fsb[64:128, par, :])
            nc.vector.tensor_mul(
                outT_t[par * 64:(par + 1) * 64, qg * 512:(qg + 1) * 512],
                fsb[0:64, par, :], rc[:])
    for u in leftovers:
        u()


def _pv(nc, fz, v_t, b, ex, kc):
    for par in range(2):
        nc.tensor.matmul(
            fz[:, par, :],
            v_t[:, par, b * 16 + kc, :],
            ex[:, par * 512:(par + 1) * 512],
            start=(kc == 0), stop=(kc == 15))


def _get_nc():
    global _cached
    if _cached is None:
        _cached = build()
    return _cached


def make_in_maps(x, w_qkv, w_out):
    bf16 = mybir.dt.np(BF16)
    x = np.asarray(x, dtype=np.float32).reshape(SB, E)
    xT = np.ascontiguousarray(x.T).astype(bf16)
    w_qkv = np.asarray(w_qkv, dtype=np.float32)
    w_out = np.asarray(w_out, dtype=np.float32).astype(bf16)
    in_maps = []
    for c in range(N_CORES):
        hs = c * HD                  # first qkv column of this core's heads
        in_maps.append({
            "xT": xT,
            "wq": np.ascontiguousarray(
                w_qkv[:, hs:hs + HD].astype(bf16)),
            "wk": np.ascontiguousarray(
                w_qkv[:, E + hs:E + hs + HD].astype(bf16)),
            "wv": np.ascontiguousarray(
                w_qkv[:, 2 * E + hs:2 * E + hs + HD].astype(bf16)),
            "wo": np.ascontiguousarray(w_out),
        })
    return in_maps


def assemble(results):
    y = np.empty((B, S, E), dtype=np.float32)
    for c in range(N_CORES):
        b, j = c // 4, c % 4
        y[b, j * 512:(j + 1) * 512, :] = results[c]["y"]
    return y


def kernel(x, w_qkv, w_out):
    nc = _get_nc()
    res = run_bass_kernel_spmd(nc, make_in_maps(x, w_qkv, w_out),
                               list(range(N_CORES)))
    return assemble(res.results)
